# revision 29
# baseline (speedup 1.0000x reference)
"""Trainium2 Bass kernel for DecoupledSOLOHead mask decoding + Matrix NMS.

Math (reference):
    mask_x = seg_preds_x[x_inds]; mask_y = seg_preds_y[y_inds]   # [N,H,W]
    soft = mask_x*mask_y; hard = soft > THR
    sum_masks = hard.sum((1,2)); seg_score = (soft*hard).sum((1,2))/max(sm,1)
    scores = cate_scores * seg_score
    inter = hard_flat @ hard_flat.T          # [N,N]
    ... matrix NMS (gaussian) -> scores * decay_coef

Strategy (8 cores), v2:
  - Shard the H*W=60800 pixel dim: 7600 px/core, zero-padded to 7680 = 60
    chunks of 128 pixels.  Slabs hold log2 of the inputs (bf16).
  - LOG-SPACE gather: log2(soft) = gather_x + gather_y accumulates in ONE
    PSUM tile via two accumulated matmuls.  DVE thresholds log2(soft) >
    log2(THR) straight from PSUM into the fp8 DoubleRow pair tile; ACT
    recovers linear soft via Exp(ln2 * ls) for the num accumulator.
  - S partials: chunks in PAIRS; 4 accumulated fp8 DoubleRow matmuls
    contract 256 pixels per pass.  v2: the moving operand is TRIMMED to
    the upper-trapezoid width 128*(m+1) per 128-row block m -- the decay
    stage only reads S[j, i<j] (plus the sm row, which lives in the full-
    width m=3 block via the constant ones column at phys col 509).
  - v2: the AllReduce carries only the trapezoid (two row-bands per block
    with widths 125m+64 / 125m+125) + num + sm: ~284KB vs 502KB.
  - Decay stage (replicated; S symmetric => transposed tiles == tiles),
    v2 restructured to avoid DRAM bounces:
      * sm broadcast down partitions via a rank-1 f32 matmul (ones
        stationary x sm row), not a DRAM partition-broadcast DMA.
      * comp^2 column->row via a single SBUF->SBUF strided DMA, squared
        once as a row, broadcast down partitions via a second rank-1
        matmul into PSUM.
      * fused DVE tensor_tensor_reduce for iou+rowmax and diff+rowmin;
        GpSimd (pool) takes the u / masked-S elementwise ops so DVE and
        GpSimd run the 4 row-tiles in parallel.
    Untouched (trimmed-away) regions of the S tiles are memset to 0 up
    front, which makes them behave exactly like label-masked pairs
    (iou=0 -> they contribute comp2_i, the reference's ratio floor).
"""

import sys

if "/opt/trn_rl_repo" not in sys.path:
    sys.path.insert(0, "/opt/trn_rl_repo")

from contextlib import ExitStack

import numpy as np
import ml_dtypes

import bass_rust
import concourse.bass as bass
import concourse.tile as tile
from concourse import bacc, mybir
from concourse.bass_utils import run_bass_kernel_spmd

N = 500
G = 128
H, W = 200, 304
HW = H * W              # 60800
NCORES = 8
PPC = HW // NCORES      # 7600 pixels per core
PAD = 7680              # padded to 60 chunks of 128
CHUNKS = PAD // 128     # 60
# slab pieces: two narrow leading pieces so chunk 0 starts ASAP
PIECES = [(0, 640), (640, 640), (1280, 1280), (2560, 1280), (3840, 1280),
          (5120, 1280), (6400, 1280)]


def _piece_of(c):
    """(piece index, col offset within piece) for chunk c"""
    col = c * 128
    for i, (base, w) in enumerate(PIECES):
        if base <= col < base + w:
            return i, col - base
    raise ValueError(c)


MT = 125                # candidate tile (4 tiles of 125 = 500)
THR = 0.005
LOG2THR = float(np.log2(THR))
LN2 = float(np.log(2.0))
SIGMA = 2.0

BF16 = mybir.dt.bfloat16
F32 = mybir.dt.float32
U16 = mybir.dt.uint16
FP8 = mybir.dt.float8e4
DR = mybir.MatmulPerfMode.DoubleRow
ALU = mybir.AluOpType
AFT = bass_rust.ActivationFunctionType

# trapezoid row-bands per 125-row block m: (row0, nrows, dense width)
TRAP = [[(0, 64, 125 * m + 64), (64, 61, 125 * m + 125)] for m in range(4)]
# two collective buffers: A = [num | sm | t0..t2 bands] (small, first so the
# decay can start on tiles 0-2 while B transfers), B = [t3 bands]
CC_NUM = 0
CC_SM = N
_offA, _offB = 2 * N, 0
TRAP_OFF = []           # (buffer, offset) of each (m, band)
for m in range(4):
    TRAP_OFF.append([])
    for (r0, nr, w) in TRAP[m]:
        if m < 3:
            TRAP_OFF[m].append(("A", _offA))
            _offA += nr * w
        else:
            TRAP_OFF[m].append(("B", _offB))
            _offB += nr * w
CC_A_LEN = _offA        # 83038
CC_B_LEN = _offB        # 58596

_NC_CACHE = []


def _r2(ap, f):
    """reshape a flat (1-D) AP slice to [p, f]"""
    return ap.rearrange("(p f) -> p f", f=f)


def _bcast(ap_flat, p, n):
    """partition-broadcast AP: read the same n elements into p partitions"""
    return bass.AP(tensor=ap_flat.tensor, offset=ap_flat.offset,
                   ap=[[0, p], [1, n]])


def _build_nc():
    nc = bacc.Bacc("TRN2", target_bir_lowering=False, debug=False,
                   num_devices=NCORES)

    xs_d = nc.dram_tensor("xs", [G, PAD], BF16, kind="ExternalInput")
    ys_d = nc.dram_tensor("ys", [G, PAD], BF16, kind="ExternalInput")
    ohx_d = nc.dram_tensor("ohx", [G, N], BF16, kind="ExternalInput")
    ohy_d = nc.dram_tensor("ohy", [G, N], BF16, kind="ExternalInput")
    # maskt[t][j_local, i] = (labels[i]==labels[125t+j_local]) & (i < 125t+j_local)
    maskt_d = nc.dram_tensor("maskt", [4, MT, N], BF16, kind="ExternalInput")
    # cate in column layout: catec[j, t] = cate_scores[125t + j]
    cate_d = nc.dram_tensor("cate", [MT, 4], F32, kind="ExternalInput")
    out_d = nc.dram_tensor("out", [N], F32, kind="ExternalOutput")

    with tile.TileContext(nc) as tc, ExitStack() as ctx:
        consts = ctx.enter_context(tc.tile_pool(name="consts", bufs=1))
        work = ctx.enter_context(tc.tile_pool(name="work", bufs=3))
        fin = ctx.enter_context(tc.tile_pool(name="fin", bufs=1))
        psS = ctx.enter_context(tc.tile_pool(name="psS", bufs=1, space="PSUM"))
        psG = ctx.enter_context(tc.tile_pool(name="psG", bufs=1, space="PSUM"))
        dram = ctx.enter_context(tc.tile_pool(name="dram", bufs=1, space="DRAM"))

        # ---- input DMAs: xs/ohx on sync, ys/ohy on scalar, rest on gpsimd;
        #      piece 0 + onehot first so chunk 0 starts ASAP ----
        xs_p = [consts.tile([G, w], BF16, name=f"xs{p}")
                for p, (_, w) in enumerate(PIECES)]
        ys_p = [consts.tile([G, w], BF16, name=f"ys{p}")
                for p, (_, w) in enumerate(PIECES)]
        ohx_s = consts.tile([G, N], BF16)
        ohy_s = consts.tile([G, N], BF16)
        nc.sync.dma_start(ohx_s[:], ohx_d[:])
        nc.sync.dma_start(ohy_s[:], ohy_d[:])
        for p, (base, w) in enumerate(PIECES):
            sl = np.s_[:, base:base + w]
            nc.sync.dma_start(xs_p[p][:], xs_d[sl])
            nc.sync.dma_start(ys_p[p][:], ys_d[sl])
        maskt_s = []
        for t in range(4):
            mt_ = consts.tile([MT, N], BF16, name=f"maskt{t}")
            nc.scalar.dma_start(mt_[:], maskt_d[t])
            maskt_s.append(mt_)
        catec = consts.tile([MT, 4], F32)
        nc.scalar.dma_start(catec[:], cate_d[:])

        # fat ones stationary for the DoubleRow num matmul (only out row 0
        # is read; M=64 because DR ldweights rejects tiny stationaries)
        ones_dr = consts.tile([G, 2, 64], FP8)
        nc.gpsimd.memset(ones_dr[:], 1.0)
        # rank-1 broadcast stationary (partition-dim 1)
        ones_f32 = consts.tile([1, MT], F32)
        nc.gpsimd.memset(ones_f32[:], 1.0)

        # decay-stage S tiles: memset now (gpsimd is idle); trimmed-away
        # columns must read as 0 so they act like label-masked pairs.
        st = []
        for t in range(4):
            s = fin.tile([MT, N], U16, name=f"st{t}")
            nc.gpsimd.memset(s[:], 0)
            st.append(s)

        # DoubleRow ldweights needs 128-aligned stationary slices, so the
        # candidate axis uses a 512-wide PHYSICAL layout: candidate
        # 125*b + c lives at physical column 128*b + c (3 pad cols per
        # block, garbage, whose matmul outputs are never read).  Physical
        # column 509 (pad of block 3) is a constant ones column, making
        # s_ps[3] row 125 = sum_masks.
        PHY = 512

        def phyv(ap2d, nb=4):
            """[P, 128*nb] AP -> [P, nb, 125] view of the valid columns"""
            return ap2d.rearrange("p (b c) -> p b c", b=nb)[:, :, 0:MT]

        # ---- PSUM: 4 trimmed S tiles + num; gather tiles in the rest ----
        # full-bank tiles (PSUM accumulation groups are bank-granular);
        # the matmuls only write the trimmed [:, 0:128*(m+1)] slice
        s_ps = [psS.tile([128, PHY], F32, name=f"s_ps{m}")
                for m in range(4)]
        num_ps = psS.tile([64, PHY], F32)


        # ---- chunk-pair loop (DoubleRow contracts 256 pixels per pass) ----
        # fp8 elementwise INPUTS are pathologically slow on DVE/GpSimd
        # (~10x), so the threshold/exp read f32 PSUM and only WRITE fp8.
        for pp in range(CHUNKS // 2):
            first, last = (pp == 0), (pp == CHUNKS // 2 - 1)
            hard2 = work.tile([128, 2, PHY], FP8, tag="hard2", bufs=2,
                              name="hard2")
            soft2 = work.tile([128, 2, PHY], FP8, tag="soft2", bufs=2,
                              name="soft2")
            for s in range(2):
                c = 2 * pp + s
                p, off = _piece_of(c)
                # LOG-SPACE gather: log2(soft) = gather_x + gather_y in ONE
                # PSUM tile via two accumulated matmuls.
                ls = psG.tile([128, N], F32, tag="ls", bufs=3, name="ls")
                xsl = xs_p[p][:, off:off + 128]
                ysl = ys_p[p][:, off:off + 128]
                nc.tensor.matmul(ls[:], xsl, ohx_s[:], start=True, stop=False)
                nc.tensor.matmul(ls[:], ysl, ohy_s[:], start=False, stop=True)

                # hard = (log2(soft) > log2(THR)), fp8 out straight from PSUM
                nc.vector.tensor_scalar(phyv(hard2[:, s, :]), ls[:], LOG2THR,
                                        None, op0=ALU.is_gt)
                # linear soft for num via ACT: 2^ls = e^(ln2 * ls)
                nc.scalar.activation(phyv(soft2[:, s, :]), ls[:], AFT.Exp,
                                     scale=LN2)
            nc.gpsimd.memset(hard2[:, :, 509:510], 1.0)
            nc.tensor.matmul(num_ps[:], ones_dr[:], soft2[:, :, :],
                             start=first, stop=last, perf_mode=DR)
            for m in range(4):
                wphy = 128 * (m + 1)
                nc.tensor.matmul(s_ps[m][:, 0:wphy],
                                 hard2[:, :, 128 * m:128 * m + 128],
                                 hard2[:, :, 0:wphy], start=first, stop=last,
                                 perf_mode=DR)

        # ---- epilogue: PSUM -> u16 with phys->dense compaction; sm = row
        #      125 of s_ps[3] (the ones-column output) ----
        # The label/triu mask is folded in HERE (S entries of masked-out
        # pairs become 0), so the decay stage needs no mask multiply: it
        # reads iou = S/u directly, 0 wherever the pair is masked.
        ssb16 = []
        for m in range(4):
            w = 125 * (m + 1)
            hi = 126 if m == 3 else 125
            s16 = fin.tile([hi, w], U16, name=f"ssb16_{m}")
            if m == 3:
                # sm row (ones-column output, row 125) stays unmasked.
                # Compute engines must start at a 32-aligned partition, so
                # plain-copy rows [96:126] FIRST; the masked convert below
                # then overwrites rows 96..124 (program order = WAW order).
                nc.scalar.copy(
                    s16[96:126, :].rearrange("p (b c) -> p b c", b=4),
                    phyv(s_ps[3][96:126, :], nb=4))
            dview = s16[0:125, :].rearrange("p (b c) -> p b c", b=m + 1)
            mview = maskt_s[m][:, 0:w].rearrange("p (b c) -> p b c", b=m + 1)
            nc.vector.tensor_tensor(dview,
                                    phyv(s_ps[m][0:125, 0:128 * (m + 1)],
                                         nb=m + 1),
                                    mview, op=ALU.mult)
            ssb16.append(s16)
        # num: +0.5 so trunc-style conversion rounds to nearest
        num16 = fin.tile([1, N], U16)
        nc.vector.tensor_scalar(num16[:], phyv(num_ps[0:1, :]), 0.5, None,
                                op0=ALU.add)

        # ---- u16 AllReduce of [S trapezoid | num | sm] ----
        cc_in_a = dram.tile([CC_A_LEN], U16)
        cc_out_a = dram.tile([CC_A_LEN], U16, addr_space="Shared")
        cc_in_b = dram.tile([CC_B_LEN], U16)
        cc_out_b = dram.tile([CC_B_LEN], U16, addr_space="Shared")
        cc_ins = {"A": cc_in_a, "B": cc_in_b}
        cc_outs = {"A": cc_out_a, "B": cc_out_b}
        dma_engs = [nc.sync, nc.scalar, nc.gpsimd]
        qi = 0
        for m in range(4):
            for b, (r0, nr, w) in enumerate(TRAP[m]):
                buf, o = TRAP_OFF[m][b]
                dma_engs[qi % 3].dma_start(_r2(cc_ins[buf][o:o + nr * w], w),
                                           ssb16[m][r0:r0 + nr, 0:w])
                qi += 1
        nc.gpsimd.dma_start(_r2(cc_in_a[CC_NUM:CC_NUM + N], N), num16[:])
        nc.sync.dma_start(_r2(cc_in_a[CC_SM:CC_SM + N], N),
                          ssb16[3][125:126, :])
        nc.gpsimd.collective_compute(
            "AllReduce", ALU.add, replica_groups=[list(range(NCORES))],
            ins=[cc_in_a.opt()], outs=[cc_out_a.opt()])
        nc.gpsimd.collective_compute(
            "AllReduce", ALU.add, replica_groups=[list(range(NCORES))],
            ins=[cc_in_b.opt()], outs=[cc_out_b.opt()])

        # ---- decay stage (replicated; S symmetric => S^T tiles == tiles) --
        # sm row -> f32 -> broadcast down partitions via rank-1 matmul
        # (faster than a 125-fold partition-broadcast DMA from DRAM)
        numsm = fin.tile([1, 2 * N], U16)
        nc.sync.dma_start(numsm[:], _r2(cc_out_a[0:2 * N], 2 * N))
        smrow = fin.tile([1, N], F32)
        nc.scalar.copy(smrow[:], numsm[0:1, N:2 * N])
        smb_t = psG.tile([128, N], F32, tag="ls", bufs=3, name="smb")
        smb_ps = smb_t[0:MT, :]
        nc.tensor.matmul(smb_ps, ones_f32[:], smrow[:], start=True,
                         stop=True)

        # num/sm columns, one single-descriptor DMA per column (a strided
        # multi-descriptor DMA lets consumers pass on the first packet);
        # tile-t inputs emitted in use order so tile 0 starts ASAP
        smc4 = fin.tile([MT, 4], U16)
        numc4 = fin.tile([MT, 4], U16)
        qi = 0
        for t in range(4):
            dma_engs[qi % 3].dma_start(
                smc4[:, t:t + 1],
                _r2(cc_out_a[CC_SM + MT * t:CC_SM + MT * (t + 1)], 1))
            qi += 1
            for b, (r0, nr, w) in enumerate(TRAP[t]):
                buf, o = TRAP_OFF[t][b]
                dma_engs[qi % 3].dma_start(
                    st[t][r0:r0 + nr, 0:w],
                    _r2(cc_outs[buf][o:o + nr * w], w))
                qi += 1
            dma_engs[qi % 3].dma_start(
                numc4[:, t:t + 1],
                _r2(cc_out_a[CC_NUM + MT * t:CC_NUM + MT * (t + 1)], 1))
            qi += 1

        # scores in column orientation: sc2[t] = cate * num / max(sm, 1)
        sc2 = []
        for t in range(4):
            smax = fin.tile([MT, 1], F32, name=f"smax{t}")
            nc.vector.tensor_scalar(smax[:], smc4[:, t:t + 1], 1.0, None,
                                    op0=ALU.max)
            rs = fin.tile([MT, 1], F32, name=f"rs{t}")
            nc.vector.reciprocal_approx_fast(rs[:], smax[:])
            s1 = fin.tile([MT, 1], F32, name=f"s1_{t}")
            nc.vector.tensor_tensor(s1[:], numc4[:, t:t + 1], rs[:],
                                    op=ALU.mult)
            s2 = fin.tile([MT, 1], F32, name=f"s2_{t}")
            nc.vector.tensor_tensor(s2[:], s1[:], catec[:, t:t + 1],
                                    op=ALU.mult)
            sc2.append(s2)

        # phase A per tile t (DVE; Pool cannot do general elementwise).
        # S is pre-masked (epilogue), so iou = S/u is already masked:
        #   u    = (sm_bcast + sm_col) - S_masked  (DVE stt, reads PSUM)
        #   ru   = 1/u                             (DVE)
        #   iou  = S * ru                          (DVE)
        #   cmax = rowmax(iou)                     (DVE reduce)
        #   sqm  = iou^2, csq = cmax^2             (scalar ACT)
        # (tensor_tensor_reduce would fuse iou+cmax but crashes on HW.)
        # NOTE: u uses the MASKED S, so u is wrong (too big) exactly where
        # iou is 0 anyway -- harmless, and u >= 1 still holds there.
        cmax4 = fin.tile([MT, 4], F32)
        sqm_t = []
        for t in range(4):
            u = work.tile([MT, N], F32, tag="u", bufs=2, name="u")
            nc.vector.scalar_tensor_tensor(u[:], smb_ps, smc4[:, t:t + 1],
                                           st[t][:], op0=ALU.add,
                                           op1=ALU.subtract)
            ru = work.tile([MT, N], F32, tag="ru", bufs=2, name="ru")
            nc.vector.reciprocal_approx_fast(ru[:], u[:])
            iou = work.tile([MT, N], F32, tag="iou", bufs=2, name="iou")
            nc.vector.tensor_tensor(iou[:], st[t][:], ru[:], op=ALU.mult)
            nc.vector.tensor_reduce(cmax4[:, t:t + 1], iou[:],
                                    axis=mybir.AxisListType.X, op=ALU.max)
            sqm = fin.tile([MT, N], F32, name=f"sqm{t}")
            nc.scalar.activation(sqm[:], iou[:], AFT.Square)
            sqm_t.append(sqm)

        # comp^2: cmax columns -> one row (4 single-descriptor SBUF->SBUF
        # DMAs), square once as a row, broadcast down via rank-1 matmul
        c2row = fin.tile([1, N], F32)
        for t in range(4):
            dma_engs[t % 3].dma_start(c2row[0:1, MT * t:MT * (t + 1)],
                                      cmax4[:, t:t + 1])
        c2sq = fin.tile([1, N], F32)
        nc.scalar.activation(c2sq[:], c2row[:], AFT.Square)
        rcb_t = psG.tile([128, N], F32, tag="ls", bufs=3, name="rcb")
        rcb = rcb_t[0:MT, :]
        nc.tensor.matmul(rcb, ones_f32[:], c2sq[:], start=True, stop=True)

        # phase B: dec[j] = exp(SIGMA * min_i(comp2_i - sqm[j,i]))
        for t in range(4):
            diff = work.tile([MT, N], F32, tag="diff", bufs=2, name="diff")
            nc.vector.tensor_tensor(diff[:], rcb, sqm_t[t][:],
                                    op=ALU.subtract)
            dcol = fin.tile([MT, 1], F32, name=f"dcol{t}")
            nc.vector.tensor_reduce(dcol[:], diff[:],
                                    axis=mybir.AxisListType.X, op=ALU.min)
            dec = fin.tile([MT, 1], F32, name=f"dec{t}")
            nc.scalar.activation(dec[:], dcol[:], AFT.Exp, scale=float(SIGMA))
            res = fin.tile([MT, 1], F32, name=f"res{t}")
            nc.vector.tensor_tensor(res[:], sc2[t][:], dec[:], op=ALU.mult)
            dma_engs[t % 3].dma_start(_r2(out_d[MT * t:MT * (t + 1)], 1),
                                      res[:])

    nc.compile()
    return nc


def _get_nc():
    if not _NC_CACHE:
        _NC_CACHE.append(_build_nc())
    return _NC_CACHE[0]


def _prep_inputs(cate_scores, seg_preds_x, seg_preds_y, cate_labels, x_inds,
                 y_inds):
    bf16 = ml_dtypes.bfloat16
    # slabs hold log2 of the inputs (log-space gather); clamp away zeros
    X = np.log2(np.maximum(
        np.asarray(seg_preds_x, np.float32).reshape(G, HW), 1e-30)).astype(bf16)
    Y = np.log2(np.maximum(
        np.asarray(seg_preds_y, np.float32).reshape(G, HW), 1e-30)).astype(bf16)

    xi = np.asarray(x_inds).astype(np.int64)
    yi = np.asarray(y_inds).astype(np.int64)
    lab = np.asarray(cate_labels).astype(np.int64)
    ohx = (np.arange(G)[:, None] == xi[None, :]).astype(bf16)
    ohy = (np.arange(G)[:, None] == yi[None, :]).astype(bf16)

    jj = np.arange(N)
    maskt = ((lab[None, :] == lab[:, None]) &
             (jj[None, :] < jj[:, None])).astype(bf16).reshape(4, MT, N)
    cate = np.ascontiguousarray(
        np.asarray(cate_scores, np.float32).reshape(4, MT).T)

    in_maps = []
    for k in range(NCORES):
        sl = np.s_[:, k * PPC:(k + 1) * PPC]
        m = {}
        for name, arr in (("xs", X), ("ys", Y)):
            # pad with a large negative log so padded pixels give soft ~ 0
            s = np.full((G, PAD), -100.0, bf16)
            s[:, :PPC] = arr[sl]
            m[name] = s
        m["ohx"] = ohx
        m["ohy"] = ohy
        m["maskt"] = maskt
        m["cate"] = cate
        in_maps.append(m)
    return in_maps


def kernel(**inputs) -> np.ndarray:
    in_maps = _prep_inputs(**inputs)
    nc = _get_nc()
    import os
    trace = bool(os.environ.get("KTRACE"))
    res = run_bass_kernel_spmd(nc, in_maps, core_ids=list(range(NCORES)),
                               trace=trace)
    if trace:
        print("exec_time_ns:", res.exec_time_ns)
    return np.asarray(res.results[0]["out"], np.float32).reshape(N)


if __name__ == "__main__":
    rng = np.random.default_rng(0)
    inputs = dict(
        cate_scores=rng.random(N, np.float32),
        seg_preds_x=rng.random((G, H, W), np.float32),
        seg_preds_y=rng.random((G, H, W), np.float32),
        cate_labels=rng.integers(0, 80, N),
        x_inds=rng.integers(0, G, N),
        y_inds=rng.integers(0, G, N),
    )
    out = kernel(**inputs)
    print(out[:10])


# revision 30
# speedup vs baseline: 1.1352x; 1.1352x over previous
"""Trainium2 Bass kernel for DecoupledSOLOHead mask decoding + Matrix NMS.

Math (reference):
    mask_x = seg_preds_x[x_inds]; mask_y = seg_preds_y[y_inds]   # [N,H,W]
    soft = mask_x*mask_y; hard = soft > THR
    sum_masks = hard.sum((1,2)); seg_score = (soft*hard).sum((1,2))/max(sm,1)
    scores = cate_scores * seg_score
    inter = hard_flat @ hard_flat.T          # [N,N]
    ... matrix NMS (gaussian) -> scores * decay_coef

Strategy (8 cores), v2:
  - Shard the H*W=60800 pixel dim: 7600 px/core, zero-padded to 7680 = 60
    chunks of 128 pixels.  Slabs hold log2 of the inputs (bf16).
  - LOG-SPACE gather: log2(soft) = gather_x + gather_y accumulates in ONE
    PSUM tile via two accumulated matmuls.  DVE thresholds log2(soft) >
    log2(THR) straight from PSUM into the fp8 DoubleRow pair tile; ACT
    recovers linear soft via Exp(ln2 * ls) for the num accumulator.
  - S partials: chunks in PAIRS; 4 accumulated fp8 DoubleRow matmuls
    contract 256 pixels per pass.  v2: the moving operand is TRIMMED to
    the upper-trapezoid width 128*(m+1) per 128-row block m -- the decay
    stage only reads S[j, i<j] (plus the sm row, which lives in the full-
    width m=3 block via the constant ones column at phys col 509).
  - v2: the AllReduce carries only the trapezoid (two row-bands per block
    with widths 125m+64 / 125m+125) + num + sm: ~284KB vs 502KB.
  - Decay stage (replicated; S symmetric => transposed tiles == tiles),
    v2 restructured to avoid DRAM bounces:
      * sm broadcast down partitions via a rank-1 f32 matmul (ones
        stationary x sm row), not a DRAM partition-broadcast DMA.
      * comp^2 column->row via a single SBUF->SBUF strided DMA, squared
        once as a row, broadcast down partitions via a second rank-1
        matmul into PSUM.
      * fused DVE tensor_tensor_reduce for iou+rowmax and diff+rowmin;
        GpSimd (pool) takes the u / masked-S elementwise ops so DVE and
        GpSimd run the 4 row-tiles in parallel.
    Untouched (trimmed-away) regions of the S tiles are memset to 0 up
    front, which makes them behave exactly like label-masked pairs
    (iou=0 -> they contribute comp2_i, the reference's ratio floor).
"""

import sys

if "/opt/trn_rl_repo" not in sys.path:
    sys.path.insert(0, "/opt/trn_rl_repo")

from contextlib import ExitStack

import numpy as np
import ml_dtypes

import bass_rust
import concourse.bass as bass
import concourse.tile as tile
from concourse import bacc, mybir
from concourse.bass_utils import run_bass_kernel_spmd

N = 500
G = 128
H, W = 200, 304
HW = H * W              # 60800
NCORES = 8
PPC = HW // NCORES      # 7600 pixels per core
PAD = 7680              # padded to 60 chunks of 128
CHUNKS = PAD // 128     # 60
# slab pieces: two narrow leading pieces so chunk 0 starts ASAP
PIECES = [(0, 640), (640, 640), (1280, 1280), (2560, 1280), (3840, 1280),
          (5120, 1280), (6400, 1280)]


def _piece_of(c):
    """(piece index, col offset within piece) for chunk c"""
    col = c * 128
    for i, (base, w) in enumerate(PIECES):
        if base <= col < base + w:
            return i, col - base
    raise ValueError(c)


MT = 125                # candidate tile (4 tiles of 125 = 500)
THR = 0.005
LOG2THR = float(np.log2(THR))
LN2 = float(np.log(2.0))
SIGMA = 2.0

BF16 = mybir.dt.bfloat16
F32 = mybir.dt.float32
U16 = mybir.dt.uint16
FP8 = mybir.dt.float8e4
DR = mybir.MatmulPerfMode.DoubleRow
ALU = mybir.AluOpType
AFT = bass_rust.ActivationFunctionType

# trapezoid row-bands per 125-row block m: (row0, nrows, dense width)
TRAP = [[(0, 64, 125 * m + 64), (64, 61, 125 * m + 125)] for m in range(4)]
CC_NUM = 0
CC_SM = N
_off = 2 * N
TRAP_OFF = []           # cc offset of each (m, band)
for m in range(4):
    TRAP_OFF.append([])
    for (r0, nr, w) in TRAP[m]:
        TRAP_OFF[m].append(_off)
        _off += nr * w
CC_LEN = _off           # 141634

_NC_CACHE = []


def _r2(ap, f):
    """reshape a flat (1-D) AP slice to [p, f]"""
    return ap.rearrange("(p f) -> p f", f=f)


def _bcast(ap_flat, p, n):
    """partition-broadcast AP: read the same n elements into p partitions"""
    return bass.AP(tensor=ap_flat.tensor, offset=ap_flat.offset,
                   ap=[[0, p], [1, n]])


def _build_nc():
    nc = bacc.Bacc("TRN2", target_bir_lowering=False, debug=False,
                   num_devices=NCORES)

    xs_d = nc.dram_tensor("xs", [G, PAD], BF16, kind="ExternalInput")
    ys_d = nc.dram_tensor("ys", [G, PAD], BF16, kind="ExternalInput")
    ohx_d = nc.dram_tensor("ohx", [G, N], BF16, kind="ExternalInput")
    ohy_d = nc.dram_tensor("ohy", [G, N], BF16, kind="ExternalInput")
    # maskt[t][j_local, i] = (labels[i]==labels[125t+j_local]) & (i < 125t+j_local)
    maskt_d = nc.dram_tensor("maskt", [4, MT, N], BF16, kind="ExternalInput")
    # cate in column layout: catec[j, t] = cate_scores[125t + j]
    cate_d = nc.dram_tensor("cate", [MT, 4], F32, kind="ExternalInput")
    out_d = nc.dram_tensor("out", [N], F32, kind="ExternalOutput")

    with tile.TileContext(nc) as tc, ExitStack() as ctx:
        consts = ctx.enter_context(tc.tile_pool(name="consts", bufs=1))
        work = ctx.enter_context(tc.tile_pool(name="work", bufs=3))
        fin = ctx.enter_context(tc.tile_pool(name="fin", bufs=1))
        psS = ctx.enter_context(tc.tile_pool(name="psS", bufs=1, space="PSUM"))
        psG = ctx.enter_context(tc.tile_pool(name="psG", bufs=1, space="PSUM"))
        dram = ctx.enter_context(tc.tile_pool(name="dram", bufs=1, space="DRAM"))

        # ---- input DMAs: xs/ohx on sync, ys/ohy on scalar, rest on gpsimd;
        #      piece 0 + onehot first so chunk 0 starts ASAP ----
        xs_p = [consts.tile([G, w], BF16, name=f"xs{p}")
                for p, (_, w) in enumerate(PIECES)]
        ys_p = [consts.tile([G, w], BF16, name=f"ys{p}")
                for p, (_, w) in enumerate(PIECES)]
        ohx_s = consts.tile([G, N], BF16)
        ohy_s = consts.tile([G, N], BF16)
        nc.sync.dma_start(ohx_s[:], ohx_d[:])
        nc.sync.dma_start(ohy_s[:], ohy_d[:])
        for p, (base, w) in enumerate(PIECES):
            sl = np.s_[:, base:base + w]
            nc.sync.dma_start(xs_p[p][:], xs_d[sl])
            nc.sync.dma_start(ys_p[p][:], ys_d[sl])
        maskt_s = []
        for t in range(4):
            mt_ = consts.tile([MT, N], BF16, name=f"maskt{t}")
            nc.scalar.dma_start(mt_[:], maskt_d[t])
            maskt_s.append(mt_)
        catec = consts.tile([MT, 4], F32)
        nc.scalar.dma_start(catec[:], cate_d[:])

        # fat ones stationary for the DoubleRow num matmul (only out row 0
        # is read; M=64 because DR ldweights rejects tiny stationaries)
        ones_dr = consts.tile([G, 2, 64], FP8)
        nc.gpsimd.memset(ones_dr[:], 1.0)
        # rank-1 broadcast stationary (partition-dim 1)
        ones_f32 = consts.tile([1, MT], F32)
        nc.gpsimd.memset(ones_f32[:], 1.0)

        # decay-stage S tiles: memset now (gpsimd is idle); trimmed-away
        # columns must read as 0 so they act like label-masked pairs.
        st = []
        for t in range(4):
            s = fin.tile([MT, N], U16, name=f"st{t}")
            nc.gpsimd.memset(s[:], 0)
            st.append(s)

        # DoubleRow ldweights needs 128-aligned stationary slices, so the
        # candidate axis uses a 512-wide PHYSICAL layout: candidate
        # 125*b + c lives at physical column 128*b + c (3 pad cols per
        # block, garbage, whose matmul outputs are never read).  Physical
        # column 509 (pad of block 3) is a constant ones column, making
        # s_ps[3] row 125 = sum_masks.
        PHY = 512

        def phyv(ap2d, nb=4):
            """[P, 128*nb] AP -> [P, nb, 125] view of the valid columns"""
            return ap2d.rearrange("p (b c) -> p b c", b=nb)[:, :, 0:MT]

        # ---- PSUM: 4 trimmed S tiles + num; gather tiles in the rest ----
        # full-bank tiles (PSUM accumulation groups are bank-granular);
        # the matmuls only write the trimmed [:, 0:128*(m+1)] slice
        s_ps = [psS.tile([128, PHY], F32, name=f"s_ps{m}")
                for m in range(4)]
        num_ps = psS.tile([64, PHY], F32)


        # ---- chunk-pair loop (DoubleRow contracts 256 pixels per pass) ----
        # fp8 elementwise INPUTS are pathologically slow on DVE/GpSimd
        # (~10x), so the threshold/exp read f32 PSUM and only WRITE fp8.
        for pp in range(CHUNKS // 2):
            first, last = (pp == 0), (pp == CHUNKS // 2 - 1)
            hard2 = work.tile([128, 2, PHY], FP8, tag="hard2", bufs=2,
                              name="hard2")
            soft2 = work.tile([128, 2, PHY], FP8, tag="soft2", bufs=2,
                              name="soft2")
            for s in range(2):
                c = 2 * pp + s
                p, off = _piece_of(c)
                # LOG-SPACE gather: log2(soft) = gather_x + gather_y in ONE
                # PSUM tile via two accumulated matmuls.
                ls = psG.tile([128, N], F32, tag="ls", bufs=3, name="ls")
                xsl = xs_p[p][:, off:off + 128]
                ysl = ys_p[p][:, off:off + 128]
                nc.tensor.matmul(ls[:], xsl, ohx_s[:], start=True, stop=False)
                nc.tensor.matmul(ls[:], ysl, ohy_s[:], start=False, stop=True)

                # hard = (log2(soft) > log2(THR)), fp8 out straight from PSUM
                nc.vector.tensor_scalar(phyv(hard2[:, s, :]), ls[:], LOG2THR,
                                        None, op0=ALU.is_gt)
                # linear soft for num via ACT: 2^ls = e^(ln2 * ls)
                nc.scalar.activation(phyv(soft2[:, s, :]), ls[:], AFT.Exp,
                                     scale=LN2)
            nc.gpsimd.memset(hard2[:, :, 509:510], 1.0)
            nc.tensor.matmul(num_ps[:], ones_dr[:], soft2[:, :, :],
                             start=first, stop=last, perf_mode=DR)
            for m in range(4):
                wphy = 128 * (m + 1)
                nc.tensor.matmul(s_ps[m][:, 0:wphy],
                                 hard2[:, :, 128 * m:128 * m + 128],
                                 hard2[:, :, 0:wphy], start=first, stop=last,
                                 perf_mode=DR)

        # ---- epilogue: PSUM -> u16 with phys->dense compaction; sm = row
        #      125 of s_ps[3] (the ones-column output) ----
        # The label/triu mask is folded in HERE (S entries of masked-out
        # pairs become 0), so the decay stage needs no mask multiply: it
        # reads iou = S/u directly, 0 wherever the pair is masked.
        ssb16 = []
        for m in range(4):
            w = 125 * (m + 1)
            hi = 126 if m == 3 else 125
            s16 = fin.tile([hi, w], U16, name=f"ssb16_{m}")
            if m == 3:
                # sm row (ones-column output, row 125) stays unmasked.
                # Compute engines must start at a 32-aligned partition, so
                # plain-copy rows [96:126] FIRST; the masked convert below
                # then overwrites rows 96..124 (program order = WAW order).
                nc.scalar.copy(
                    s16[96:126, :].rearrange("p (b c) -> p b c", b=4),
                    phyv(s_ps[3][96:126, :], nb=4))
            dview = s16[0:125, :].rearrange("p (b c) -> p b c", b=m + 1)
            mview = maskt_s[m][:, 0:w].rearrange("p (b c) -> p b c", b=m + 1)
            nc.vector.tensor_tensor(dview,
                                    phyv(s_ps[m][0:125, 0:128 * (m + 1)],
                                         nb=m + 1),
                                    mview, op=ALU.mult)
            ssb16.append(s16)
        # num: +0.5 so trunc-style conversion rounds to nearest
        num16 = fin.tile([1, N], U16)
        nc.vector.tensor_scalar(num16[:], phyv(num_ps[0:1, :]), 0.5, None,
                                op0=ALU.add)

        # ---- u16 AllReduce of [S trapezoid | num | sm] ----
        cc_in = dram.tile([CC_LEN], U16)
        cc_out = dram.tile([CC_LEN], U16, addr_space="Shared")
        # warm-up: a tiny AllReduce triggered as soon as the gpsimd queue
        # drains the main loop.  The real AllReduce is then queued while
        # this one runs, which skips the ~12us CC setup + slow first-op
        # transfer (measured: a queued second op runs ~2x faster).
        warm_in = dram.tile([MT], F32)
        warm_out = dram.tile([MT], F32, addr_space="Shared")
        nc.gpsimd.dma_start(_r2(warm_in[:], MT), ones_f32[:])
        nc.gpsimd.collective_compute(
            "AllReduce", ALU.add, replica_groups=[list(range(NCORES))],
            ins=[warm_in.opt()], outs=[warm_out.opt()])
        dma_engs = [nc.sync, nc.scalar, nc.gpsimd]
        qi = 0
        for m in range(4):
            for b, (r0, nr, w) in enumerate(TRAP[m]):
                o = TRAP_OFF[m][b]
                dma_engs[qi % 3].dma_start(_r2(cc_in[o:o + nr * w], w),
                                           ssb16[m][r0:r0 + nr, 0:w])
                qi += 1
        nc.gpsimd.dma_start(_r2(cc_in[CC_NUM:CC_NUM + N], N), num16[:])
        nc.sync.dma_start(_r2(cc_in[CC_SM:CC_SM + N], N),
                          ssb16[3][125:126, :])
        nc.gpsimd.collective_compute(
            "AllReduce", ALU.add, replica_groups=[list(range(NCORES))],
            ins=[cc_in.opt()], outs=[cc_out.opt()])

        # ---- decay stage (replicated; S symmetric => S^T tiles == tiles) --
        # sm row -> f32 -> broadcast down partitions via rank-1 matmul
        # (faster than a 125-fold partition-broadcast DMA from DRAM)
        numsm = fin.tile([1, 2 * N], U16)
        nc.sync.dma_start(numsm[:], _r2(cc_out[0:2 * N], 2 * N))
        smrow = fin.tile([1, N], F32)
        nc.scalar.copy(smrow[:], numsm[0:1, N:2 * N])
        smb_t = psG.tile([128, N], F32, tag="ls", bufs=3, name="smb")
        smb_ps = smb_t[0:MT, :]
        nc.tensor.matmul(smb_ps, ones_f32[:], smrow[:], start=True,
                         stop=True)

        # num/sm columns, one single-descriptor DMA per column (a strided
        # multi-descriptor DMA lets consumers pass on the first packet);
        # tile-t inputs emitted in use order so tile 0 starts ASAP
        smc4 = fin.tile([MT, 4], U16)
        numc4 = fin.tile([MT, 4], U16)
        qi = 0
        for t in range(4):
            dma_engs[qi % 3].dma_start(
                smc4[:, t:t + 1],
                _r2(cc_out[CC_SM + MT * t:CC_SM + MT * (t + 1)], 1))
            qi += 1
            for b, (r0, nr, w) in enumerate(TRAP[t]):
                o = TRAP_OFF[t][b]
                dma_engs[qi % 3].dma_start(
                    st[t][r0:r0 + nr, 0:w], _r2(cc_out[o:o + nr * w], w))
                qi += 1
            dma_engs[qi % 3].dma_start(
                numc4[:, t:t + 1],
                _r2(cc_out[CC_NUM + MT * t:CC_NUM + MT * (t + 1)], 1))
            qi += 1

        # scores in column orientation: sc2[t] = cate * num / max(sm, 1)
        sc2 = []
        for t in range(4):
            smax = fin.tile([MT, 1], F32, name=f"smax{t}")
            nc.vector.tensor_scalar(smax[:], smc4[:, t:t + 1], 1.0, None,
                                    op0=ALU.max)
            rs = fin.tile([MT, 1], F32, name=f"rs{t}")
            nc.vector.reciprocal_approx_fast(rs[:], smax[:])
            s1 = fin.tile([MT, 1], F32, name=f"s1_{t}")
            nc.vector.tensor_tensor(s1[:], numc4[:, t:t + 1], rs[:],
                                    op=ALU.mult)
            s2 = fin.tile([MT, 1], F32, name=f"s2_{t}")
            nc.vector.tensor_tensor(s2[:], s1[:], catec[:, t:t + 1],
                                    op=ALU.mult)
            sc2.append(s2)

        # phase A per tile t (DVE; Pool cannot do general elementwise).
        # S is pre-masked (epilogue), so iou = S/u is already masked:
        #   u    = (sm_bcast + sm_col) - S_masked  (DVE stt, reads PSUM)
        #   ru   = 1/u                             (DVE)
        #   iou  = S * ru                          (DVE)
        #   cmax = rowmax(iou)                     (DVE reduce)
        #   sqm  = iou^2, csq = cmax^2             (scalar ACT)
        # (tensor_tensor_reduce would fuse iou+cmax but crashes on HW.)
        # NOTE: u uses the MASKED S, so u is wrong (too big) exactly where
        # iou is 0 anyway -- harmless, and u >= 1 still holds there.
        cmax4 = fin.tile([MT, 4], F32)
        sqm_t = []
        for t in range(4):
            u = work.tile([MT, N], F32, tag="u", bufs=2, name="u")
            nc.vector.scalar_tensor_tensor(u[:], smb_ps, smc4[:, t:t + 1],
                                           st[t][:], op0=ALU.add,
                                           op1=ALU.subtract)
            ru = work.tile([MT, N], F32, tag="ru", bufs=2, name="ru")
            nc.vector.reciprocal_approx_fast(ru[:], u[:])
            iou = work.tile([MT, N], F32, tag="iou", bufs=2, name="iou")
            nc.vector.tensor_tensor(iou[:], st[t][:], ru[:], op=ALU.mult)
            nc.vector.tensor_reduce(cmax4[:, t:t + 1], iou[:],
                                    axis=mybir.AxisListType.X, op=ALU.max)
            sqm = fin.tile([MT, N], F32, name=f"sqm{t}")
            nc.scalar.activation(sqm[:], iou[:], AFT.Square)
            sqm_t.append(sqm)

        # comp^2: cmax columns -> one row (4 single-descriptor SBUF->SBUF
        # DMAs), square once as a row, broadcast down via rank-1 matmul
        c2row = fin.tile([1, N], F32)
        for t in range(4):
            dma_engs[t % 3].dma_start(c2row[0:1, MT * t:MT * (t + 1)],
                                      cmax4[:, t:t + 1])
        c2sq = fin.tile([1, N], F32)
        nc.scalar.activation(c2sq[:], c2row[:], AFT.Square)
        rcb_t = psG.tile([128, N], F32, tag="ls", bufs=3, name="rcb")
        rcb = rcb_t[0:MT, :]
        nc.tensor.matmul(rcb, ones_f32[:], c2sq[:], start=True, stop=True)

        # phase B: dec[j] = exp(SIGMA * min_i(comp2_i - sqm[j,i]))
        for t in range(4):
            diff = work.tile([MT, N], F32, tag="diff", bufs=2, name="diff")
            nc.vector.tensor_tensor(diff[:], rcb, sqm_t[t][:],
                                    op=ALU.subtract)
            dcol = fin.tile([MT, 1], F32, name=f"dcol{t}")
            nc.vector.tensor_reduce(dcol[:], diff[:],
                                    axis=mybir.AxisListType.X, op=ALU.min)
            dec = fin.tile([MT, 1], F32, name=f"dec{t}")
            nc.scalar.activation(dec[:], dcol[:], AFT.Exp, scale=float(SIGMA))
            res = fin.tile([MT, 1], F32, name=f"res{t}")
            nc.vector.tensor_tensor(res[:], sc2[t][:], dec[:], op=ALU.mult)
            dma_engs[t % 3].dma_start(_r2(out_d[MT * t:MT * (t + 1)], 1),
                                      res[:])

    nc.compile()
    return nc


def _get_nc():
    if not _NC_CACHE:
        _NC_CACHE.append(_build_nc())
    return _NC_CACHE[0]


def _prep_inputs(cate_scores, seg_preds_x, seg_preds_y, cate_labels, x_inds,
                 y_inds):
    bf16 = ml_dtypes.bfloat16
    # slabs hold log2 of the inputs (log-space gather); clamp away zeros
    X = np.log2(np.maximum(
        np.asarray(seg_preds_x, np.float32).reshape(G, HW), 1e-30)).astype(bf16)
    Y = np.log2(np.maximum(
        np.asarray(seg_preds_y, np.float32).reshape(G, HW), 1e-30)).astype(bf16)

    xi = np.asarray(x_inds).astype(np.int64)
    yi = np.asarray(y_inds).astype(np.int64)
    lab = np.asarray(cate_labels).astype(np.int64)
    ohx = (np.arange(G)[:, None] == xi[None, :]).astype(bf16)
    ohy = (np.arange(G)[:, None] == yi[None, :]).astype(bf16)

    jj = np.arange(N)
    maskt = ((lab[None, :] == lab[:, None]) &
             (jj[None, :] < jj[:, None])).astype(bf16).reshape(4, MT, N)
    cate = np.ascontiguousarray(
        np.asarray(cate_scores, np.float32).reshape(4, MT).T)

    in_maps = []
    for k in range(NCORES):
        sl = np.s_[:, k * PPC:(k + 1) * PPC]
        m = {}
        for name, arr in (("xs", X), ("ys", Y)):
            # pad with a large negative log so padded pixels give soft ~ 0
            s = np.full((G, PAD), -100.0, bf16)
            s[:, :PPC] = arr[sl]
            m[name] = s
        m["ohx"] = ohx
        m["ohy"] = ohy
        m["maskt"] = maskt
        m["cate"] = cate
        in_maps.append(m)
    return in_maps


def kernel(**inputs) -> np.ndarray:
    in_maps = _prep_inputs(**inputs)
    nc = _get_nc()
    import os
    trace = bool(os.environ.get("KTRACE"))
    res = run_bass_kernel_spmd(nc, in_maps, core_ids=list(range(NCORES)),
                               trace=trace)
    if trace:
        print("exec_time_ns:", res.exec_time_ns)
    return np.asarray(res.results[0]["out"], np.float32).reshape(N)


if __name__ == "__main__":
    rng = np.random.default_rng(0)
    inputs = dict(
        cate_scores=rng.random(N, np.float32),
        seg_preds_x=rng.random((G, H, W), np.float32),
        seg_preds_y=rng.random((G, H, W), np.float32),
        cate_labels=rng.integers(0, 80, N),
        x_inds=rng.integers(0, G, N),
        y_inds=rng.integers(0, G, N),
    )
    out = kernel(**inputs)
    print(out[:10])


# revision 31
# speedup vs baseline: 1.1982x; 1.0555x over previous
"""Trainium2 Bass kernel for DecoupledSOLOHead mask decoding + Matrix NMS.

Math (reference):
    mask_x = seg_preds_x[x_inds]; mask_y = seg_preds_y[y_inds]   # [N,H,W]
    soft = mask_x*mask_y; hard = soft > THR
    sum_masks = hard.sum((1,2)); seg_score = (soft*hard).sum((1,2))/max(sm,1)
    scores = cate_scores * seg_score
    inter = hard_flat @ hard_flat.T          # [N,N]
    ... matrix NMS (gaussian) -> scores * decay_coef

Strategy (8 cores), v2:
  - Shard the H*W=60800 pixel dim: 7600 px/core, zero-padded to 7680 = 60
    chunks of 128 pixels.  Slabs hold log2 of the inputs (bf16).
  - LOG-SPACE gather: log2(soft) = gather_x + gather_y accumulates in ONE
    PSUM tile via two accumulated matmuls.  DVE thresholds log2(soft) >
    log2(THR) straight from PSUM into the fp8 DoubleRow pair tile; ACT
    recovers linear soft via Exp(ln2 * ls) for the num accumulator.
  - S partials: chunks in PAIRS; 4 accumulated fp8 DoubleRow matmuls
    contract 256 pixels per pass.  v2: the moving operand is TRIMMED to
    the upper-trapezoid width 128*(m+1) per 128-row block m -- the decay
    stage only reads S[j, i<j] (plus the sm row, which lives in the full-
    width m=3 block via the constant ones column at phys col 509).
  - v2: the AllReduce carries only the trapezoid (two row-bands per block
    with widths 125m+64 / 125m+125) + num + sm: ~284KB vs 502KB.
  - Decay stage (replicated; S symmetric => transposed tiles == tiles),
    v2 restructured to avoid DRAM bounces:
      * sm broadcast down partitions via a rank-1 f32 matmul (ones
        stationary x sm row), not a DRAM partition-broadcast DMA.
      * comp^2 column->row via a single SBUF->SBUF strided DMA, squared
        once as a row, broadcast down partitions via a second rank-1
        matmul into PSUM.
      * fused DVE tensor_tensor_reduce for iou+rowmax and diff+rowmin;
        GpSimd (pool) takes the u / masked-S elementwise ops so DVE and
        GpSimd run the 4 row-tiles in parallel.
    Untouched (trimmed-away) regions of the S tiles are memset to 0 up
    front, which makes them behave exactly like label-masked pairs
    (iou=0 -> they contribute comp2_i, the reference's ratio floor).
"""

import sys

if "/opt/trn_rl_repo" not in sys.path:
    sys.path.insert(0, "/opt/trn_rl_repo")

from contextlib import ExitStack

import numpy as np
import ml_dtypes

import bass_rust
import concourse.bass as bass
import concourse.tile as tile
from concourse import bacc, mybir
from concourse.bass_utils import run_bass_kernel_spmd

N = 500
G = 128
H, W = 200, 304
HW = H * W              # 60800
NCORES = 8
PPC = HW // NCORES      # 7600 pixels per core
PAD = 7680              # padded to 60 chunks of 128
CHUNKS = PAD // 128     # 60
# slab pieces: two narrow leading pieces so chunk 0 starts ASAP
PIECES = [(0, 640), (640, 640), (1280, 1280), (2560, 1280), (3840, 1280),
          (5120, 1280), (6400, 1280)]


def _piece_of(c):
    """(piece index, col offset within piece) for chunk c"""
    col = c * 128
    for i, (base, w) in enumerate(PIECES):
        if base <= col < base + w:
            return i, col - base
    raise ValueError(c)


MT = 125                # candidate tile (4 tiles of 125 = 500)
THR = 0.005
LOG2THR = float(np.log2(THR))
LN2 = float(np.log(2.0))
SIGMA = 2.0

BF16 = mybir.dt.bfloat16
F32 = mybir.dt.float32
U16 = mybir.dt.uint16
FP8 = mybir.dt.float8e4
DR = mybir.MatmulPerfMode.DoubleRow
ALU = mybir.AluOpType
AFT = bass_rust.ActivationFunctionType

# trapezoid row-bands per 125-row block m: (row0, nrows, dense width)
TRAP = [[(0, 64, 125 * m + 64), (64, 61, 125 * m + 125)] for m in range(4)]
# two collective buffers: A = [num | sm | t0..t2 bands] (small, first, so
# the decay can start on tiles 0-2 while B = [t3 bands] transfers)
CC_NUM = 0
CC_SM = N
_offA, _offB = 2 * N, 0
TRAP_OFF = []           # (buffer, offset) of each (m, band)
for m in range(4):
    TRAP_OFF.append([])
    for (r0, nr, w) in TRAP[m]:
        if m < 3:
            TRAP_OFF[m].append(("A", _offA))
            _offA += nr * w
        else:
            TRAP_OFF[m].append(("B", _offB))
            _offB += nr * w
CC_A_LEN = _offA        # 83038
CC_B_LEN = _offB        # 58596

_NC_CACHE = []


def _r2(ap, f):
    """reshape a flat (1-D) AP slice to [p, f]"""
    return ap.rearrange("(p f) -> p f", f=f)


def _bcast(ap_flat, p, n):
    """partition-broadcast AP: read the same n elements into p partitions"""
    return bass.AP(tensor=ap_flat.tensor, offset=ap_flat.offset,
                   ap=[[0, p], [1, n]])


def _build_nc():
    nc = bacc.Bacc("TRN2", target_bir_lowering=False, debug=False,
                   num_devices=NCORES)

    xs_d = nc.dram_tensor("xs", [G, PAD], BF16, kind="ExternalInput")
    ys_d = nc.dram_tensor("ys", [G, PAD], BF16, kind="ExternalInput")
    ohx_d = nc.dram_tensor("ohx", [G, N], BF16, kind="ExternalInput")
    ohy_d = nc.dram_tensor("ohy", [G, N], BF16, kind="ExternalInput")
    # maskt[t][j_local, i] = (labels[i]==labels[125t+j_local]) & (i < 125t+j_local)
    maskt_d = nc.dram_tensor("maskt", [4, MT, N], BF16, kind="ExternalInput")
    # cate in column layout: catec[j, t] = cate_scores[125t + j]
    cate_d = nc.dram_tensor("cate", [MT, 4], F32, kind="ExternalInput")
    out_d = nc.dram_tensor("out", [N], F32, kind="ExternalOutput")

    with tile.TileContext(nc) as tc, ExitStack() as ctx:
        consts = ctx.enter_context(tc.tile_pool(name="consts", bufs=1))
        work = ctx.enter_context(tc.tile_pool(name="work", bufs=3))
        fin = ctx.enter_context(tc.tile_pool(name="fin", bufs=1))
        psS = ctx.enter_context(tc.tile_pool(name="psS", bufs=1, space="PSUM"))
        psG = ctx.enter_context(tc.tile_pool(name="psG", bufs=1, space="PSUM"))
        dram = ctx.enter_context(tc.tile_pool(name="dram", bufs=1, space="DRAM"))

        # ---- input DMAs: xs/ohx on sync, ys/ohy on scalar, rest on gpsimd;
        #      piece 0 + onehot first so chunk 0 starts ASAP ----
        xs_p = [consts.tile([G, w], BF16, name=f"xs{p}")
                for p, (_, w) in enumerate(PIECES)]
        ys_p = [consts.tile([G, w], BF16, name=f"ys{p}")
                for p, (_, w) in enumerate(PIECES)]
        ohx_s = consts.tile([G, N], BF16)
        ohy_s = consts.tile([G, N], BF16)
        nc.sync.dma_start(ohx_s[:], ohx_d[:])
        nc.sync.dma_start(ohy_s[:], ohy_d[:])
        for p, (base, w) in enumerate(PIECES):
            sl = np.s_[:, base:base + w]
            nc.sync.dma_start(xs_p[p][:], xs_d[sl])
            nc.sync.dma_start(ys_p[p][:], ys_d[sl])
        maskt_s = []
        for t in range(4):
            mt_ = consts.tile([MT, N], BF16, name=f"maskt{t}")
            nc.scalar.dma_start(mt_[:], maskt_d[t])
            maskt_s.append(mt_)
        catec = consts.tile([MT, 4], F32)
        nc.scalar.dma_start(catec[:], cate_d[:])

        # fat ones stationary for the DoubleRow num matmul (only out row 0
        # is read; M=64 because DR ldweights rejects tiny stationaries)
        ones_dr = consts.tile([G, 2, 64], FP8)
        nc.gpsimd.memset(ones_dr[:], 1.0)
        # rank-1 broadcast stationary (partition-dim 1)
        ones_f32 = consts.tile([1, MT], F32)
        nc.gpsimd.memset(ones_f32[:], 1.0)

        # decay-stage S tiles: memset now (gpsimd is idle); trimmed-away
        # columns must read as 0 so they act like label-masked pairs.
        st = []
        for t in range(4):
            s = fin.tile([MT, N], U16, name=f"st{t}")
            nc.gpsimd.memset(s[:], 0)
            st.append(s)

        # DoubleRow ldweights needs 128-aligned stationary slices, so the
        # candidate axis uses a 512-wide PHYSICAL layout: candidate
        # 125*b + c lives at physical column 128*b + c (3 pad cols per
        # block, garbage, whose matmul outputs are never read).  Physical
        # column 509 (pad of block 3) is a constant ones column, making
        # s_ps[3] row 125 = sum_masks.
        PHY = 512

        def phyv(ap2d, nb=4):
            """[P, 128*nb] AP -> [P, nb, 125] view of the valid columns"""
            return ap2d.rearrange("p (b c) -> p b c", b=nb)[:, :, 0:MT]

        # ---- PSUM: 4 trimmed S tiles + num; gather tiles in the rest ----
        # full-bank tiles (PSUM accumulation groups are bank-granular);
        # the matmuls only write the trimmed [:, 0:128*(m+1)] slice
        s_ps = [psS.tile([128, PHY], F32, name=f"s_ps{m}")
                for m in range(4)]
        num_ps = psS.tile([64, PHY], F32)


        # ---- chunk-pair loop (DoubleRow contracts 256 pixels per pass) ----
        # fp8 elementwise INPUTS are pathologically slow on DVE/GpSimd
        # (~10x), so the threshold/exp read f32 PSUM and only WRITE fp8.
        for pp in range(CHUNKS // 2):
            first, last = (pp == 0), (pp == CHUNKS // 2 - 1)
            hard2 = work.tile([128, 2, PHY], FP8, tag="hard2", bufs=2,
                              name="hard2")
            soft2 = work.tile([128, 2, PHY], FP8, tag="soft2", bufs=2,
                              name="soft2")
            for s in range(2):
                c = 2 * pp + s
                p, off = _piece_of(c)
                # LOG-SPACE gather: log2(soft) = gather_x + gather_y in ONE
                # PSUM tile via two accumulated matmuls.
                ls = psG.tile([128, N], F32, tag="ls", bufs=3, name="ls")
                xsl = xs_p[p][:, off:off + 128]
                ysl = ys_p[p][:, off:off + 128]
                nc.tensor.matmul(ls[:], xsl, ohx_s[:], start=True, stop=False)
                nc.tensor.matmul(ls[:], ysl, ohy_s[:], start=False, stop=True)

                # hard = (log2(soft) > log2(THR)), fp8 out straight from PSUM
                nc.vector.tensor_scalar(phyv(hard2[:, s, :]), ls[:], LOG2THR,
                                        None, op0=ALU.is_gt)
                # linear soft for num via ACT: 2^ls = e^(ln2 * ls)
                nc.scalar.activation(phyv(soft2[:, s, :]), ls[:], AFT.Exp,
                                     scale=LN2)
            nc.gpsimd.memset(hard2[:, :, 509:510], 1.0)
            nc.tensor.matmul(num_ps[:], ones_dr[:], soft2[:, :, :],
                             start=first, stop=last, perf_mode=DR)
            for m in range(4):
                wphy = 128 * (m + 1)
                nc.tensor.matmul(s_ps[m][:, 0:wphy],
                                 hard2[:, :, 128 * m:128 * m + 128],
                                 hard2[:, :, 0:wphy], start=first, stop=last,
                                 perf_mode=DR)

        # ---- epilogue: PSUM -> u16 with phys->dense compaction; sm = row
        #      125 of s_ps[3] (the ones-column output) ----
        # The label/triu mask is folded in HERE (S entries of masked-out
        # pairs become 0), so the decay stage needs no mask multiply: it
        # reads iou = S/u directly, 0 wherever the pair is masked.
        ssb16 = []
        for m in range(4):
            w = 125 * (m + 1)
            hi = 126 if m == 3 else 125
            s16 = fin.tile([hi, w], U16, name=f"ssb16_{m}")
            if m == 3:
                # sm row (ones-column output, row 125) stays unmasked.
                # Compute engines must start at a 32-aligned partition, so
                # plain-copy rows [96:126] FIRST; the masked convert below
                # then overwrites rows 96..124 (program order = WAW order).
                nc.scalar.copy(
                    s16[96:126, :].rearrange("p (b c) -> p b c", b=4),
                    phyv(s_ps[3][96:126, :], nb=4))
            dview = s16[0:125, :].rearrange("p (b c) -> p b c", b=m + 1)
            mview = maskt_s[m][:, 0:w].rearrange("p (b c) -> p b c", b=m + 1)
            nc.vector.tensor_tensor(dview,
                                    phyv(s_ps[m][0:125, 0:128 * (m + 1)],
                                         nb=m + 1),
                                    mview, op=ALU.mult)
            ssb16.append(s16)
        # num: +0.5 so trunc-style conversion rounds to nearest
        num16 = fin.tile([1, N], U16)
        nc.vector.tensor_scalar(num16[:], phyv(num_ps[0:1, :]), 0.5, None,
                                op0=ALU.add)

        # ---- u16 AllReduce of [S trapezoid | num | sm] ----
        cc_in_a = dram.tile([CC_A_LEN], U16)
        cc_out_a = dram.tile([CC_A_LEN], U16, addr_space="Shared")
        cc_in_b = dram.tile([CC_B_LEN], U16)
        cc_out_b = dram.tile([CC_B_LEN], U16, addr_space="Shared")
        cc_ins = {"A": cc_in_a, "B": cc_in_b}
        cc_outs = {"A": cc_out_a, "B": cc_out_b}
        # warm-up: a tiny AllReduce triggered as soon as the gpsimd queue
        # drains the main loop.  The real AllReduce is then queued while
        # this one runs, which skips the ~12us CC setup + slow first-op
        # transfer (measured: a queued second op runs ~2x faster).
        warm_in = dram.tile([MT], F32)
        warm_out = dram.tile([MT], F32, addr_space="Shared")
        nc.gpsimd.dma_start(_r2(warm_in[:], MT), ones_f32[:])
        nc.gpsimd.collective_compute(
            "AllReduce", ALU.add, replica_groups=[list(range(NCORES))],
            ins=[warm_in.opt()], outs=[warm_out.opt()])
        dma_engs = [nc.sync, nc.scalar, nc.gpsimd]
        qi = 0
        for m in range(4):
            for b, (r0, nr, w) in enumerate(TRAP[m]):
                buf, o = TRAP_OFF[m][b]
                dma_engs[qi % 3].dma_start(_r2(cc_ins[buf][o:o + nr * w], w),
                                           ssb16[m][r0:r0 + nr, 0:w])
                qi += 1
        nc.gpsimd.dma_start(_r2(cc_in_a[CC_NUM:CC_NUM + N], N), num16[:])
        nc.sync.dma_start(_r2(cc_in_a[CC_SM:CC_SM + N], N),
                          ssb16[3][125:126, :])
        nc.gpsimd.collective_compute(
            "AllReduce", ALU.add, replica_groups=[list(range(NCORES))],
            ins=[cc_in_a.opt()], outs=[cc_out_a.opt()])
        nc.gpsimd.collective_compute(
            "AllReduce", ALU.add, replica_groups=[list(range(NCORES))],
            ins=[cc_in_b.opt()], outs=[cc_out_b.opt()])

        # ---- decay stage (replicated; S symmetric => S^T tiles == tiles) --
        # sm row -> f32 -> broadcast down partitions via rank-1 matmul
        # (faster than a 125-fold partition-broadcast DMA from DRAM)
        numsm = fin.tile([1, 2 * N], U16)
        nc.sync.dma_start(numsm[:], _r2(cc_out_a[0:2 * N], 2 * N))
        smrow = fin.tile([1, N], F32)
        nc.scalar.copy(smrow[:], numsm[0:1, N:2 * N])
        smb_t = psG.tile([128, N], F32, tag="ls", bufs=3, name="smb")
        smb_ps = smb_t[0:MT, :]
        nc.tensor.matmul(smb_ps, ones_f32[:], smrow[:], start=True,
                         stop=True)

        # num/sm columns, one single-descriptor DMA per column (a strided
        # multi-descriptor DMA lets consumers pass on the first packet);
        # tile-t inputs emitted in use order so tile 0 starts ASAP
        smc4 = fin.tile([MT, 4], U16)
        numc4 = fin.tile([MT, 4], U16)
        qi = 0
        for t in range(4):
            dma_engs[qi % 3].dma_start(
                smc4[:, t:t + 1],
                _r2(cc_out_a[CC_SM + MT * t:CC_SM + MT * (t + 1)], 1))
            qi += 1
            for b, (r0, nr, w) in enumerate(TRAP[t]):
                buf, o = TRAP_OFF[t][b]
                dma_engs[qi % 3].dma_start(
                    st[t][r0:r0 + nr, 0:w],
                    _r2(cc_outs[buf][o:o + nr * w], w))
                qi += 1
            dma_engs[qi % 3].dma_start(
                numc4[:, t:t + 1],
                _r2(cc_out_a[CC_NUM + MT * t:CC_NUM + MT * (t + 1)], 1))
            qi += 1

        # scores in column orientation: sc2[t] = cate * num / max(sm, 1)
        sc2 = []
        for t in range(4):
            smax = fin.tile([MT, 1], F32, name=f"smax{t}")
            nc.vector.tensor_scalar(smax[:], smc4[:, t:t + 1], 1.0, None,
                                    op0=ALU.max)
            rs = fin.tile([MT, 1], F32, name=f"rs{t}")
            nc.vector.reciprocal_approx_fast(rs[:], smax[:])
            s1 = fin.tile([MT, 1], F32, name=f"s1_{t}")
            nc.vector.tensor_tensor(s1[:], numc4[:, t:t + 1], rs[:],
                                    op=ALU.mult)
            s2 = fin.tile([MT, 1], F32, name=f"s2_{t}")
            nc.vector.tensor_tensor(s2[:], s1[:], catec[:, t:t + 1],
                                    op=ALU.mult)
            sc2.append(s2)

        # phase A per tile t (DVE; Pool cannot do general elementwise).
        # S is pre-masked (epilogue), so iou = S/u is already masked:
        #   u    = (sm_bcast + sm_col) - S_masked  (DVE stt, reads PSUM)
        #   ru   = 1/u                             (DVE)
        #   iou  = S * ru                          (DVE)
        #   cmax = rowmax(iou)                     (DVE reduce)
        #   sqm  = iou^2, csq = cmax^2             (scalar ACT)
        # (tensor_tensor_reduce would fuse iou+cmax but crashes on HW.)
        # NOTE: u uses the MASKED S, so u is wrong (too big) exactly where
        # iou is 0 anyway -- harmless, and u >= 1 still holds there.
        cmax4 = fin.tile([MT, 4], F32)
        sqm_t = []
        for t in range(4):
            u = work.tile([MT, N], F32, tag="u", bufs=2, name="u")
            nc.vector.scalar_tensor_tensor(u[:], smb_ps, smc4[:, t:t + 1],
                                           st[t][:], op0=ALU.add,
                                           op1=ALU.subtract)
            ru = work.tile([MT, N], F32, tag="ru", bufs=2, name="ru")
            nc.vector.reciprocal_approx_fast(ru[:], u[:])
            iou = work.tile([MT, N], F32, tag="iou", bufs=2, name="iou")
            nc.vector.tensor_tensor(iou[:], st[t][:], ru[:], op=ALU.mult)
            nc.vector.tensor_reduce(cmax4[:, t:t + 1], iou[:],
                                    axis=mybir.AxisListType.X, op=ALU.max)
            sqm = fin.tile([MT, N], F32, name=f"sqm{t}")
            nc.scalar.activation(sqm[:], iou[:], AFT.Square)
            sqm_t.append(sqm)

        # comp^2: cmax columns -> one row (4 single-descriptor SBUF->SBUF
        # DMAs), square once as a row, broadcast down via rank-1 matmul
        c2row = fin.tile([1, N], F32)
        for t in range(4):
            dma_engs[t % 3].dma_start(c2row[0:1, MT * t:MT * (t + 1)],
                                      cmax4[:, t:t + 1])
        c2sq = fin.tile([1, N], F32)
        nc.scalar.activation(c2sq[:], c2row[:], AFT.Square)
        rcb_t = psG.tile([128, N], F32, tag="ls", bufs=3, name="rcb")
        rcb = rcb_t[0:MT, :]
        nc.tensor.matmul(rcb, ones_f32[:], c2sq[:], start=True, stop=True)

        # phase B: dec[j] = exp(SIGMA * min_i(comp2_i - sqm[j,i]))
        for t in range(4):
            diff = work.tile([MT, N], F32, tag="diff", bufs=2, name="diff")
            nc.vector.tensor_tensor(diff[:], rcb, sqm_t[t][:],
                                    op=ALU.subtract)
            dcol = fin.tile([MT, 1], F32, name=f"dcol{t}")
            nc.vector.tensor_reduce(dcol[:], diff[:],
                                    axis=mybir.AxisListType.X, op=ALU.min)
            dec = fin.tile([MT, 1], F32, name=f"dec{t}")
            nc.scalar.activation(dec[:], dcol[:], AFT.Exp, scale=float(SIGMA))
            res = fin.tile([MT, 1], F32, name=f"res{t}")
            nc.vector.tensor_tensor(res[:], sc2[t][:], dec[:], op=ALU.mult)
            dma_engs[t % 3].dma_start(_r2(out_d[MT * t:MT * (t + 1)], 1),
                                      res[:])

    nc.compile()
    return nc


def _get_nc():
    if not _NC_CACHE:
        _NC_CACHE.append(_build_nc())
    return _NC_CACHE[0]


def _prep_inputs(cate_scores, seg_preds_x, seg_preds_y, cate_labels, x_inds,
                 y_inds):
    bf16 = ml_dtypes.bfloat16
    # slabs hold log2 of the inputs (log-space gather); clamp away zeros
    X = np.log2(np.maximum(
        np.asarray(seg_preds_x, np.float32).reshape(G, HW), 1e-30)).astype(bf16)
    Y = np.log2(np.maximum(
        np.asarray(seg_preds_y, np.float32).reshape(G, HW), 1e-30)).astype(bf16)

    xi = np.asarray(x_inds).astype(np.int64)
    yi = np.asarray(y_inds).astype(np.int64)
    lab = np.asarray(cate_labels).astype(np.int64)
    ohx = (np.arange(G)[:, None] == xi[None, :]).astype(bf16)
    ohy = (np.arange(G)[:, None] == yi[None, :]).astype(bf16)

    jj = np.arange(N)
    maskt = ((lab[None, :] == lab[:, None]) &
             (jj[None, :] < jj[:, None])).astype(bf16).reshape(4, MT, N)
    cate = np.ascontiguousarray(
        np.asarray(cate_scores, np.float32).reshape(4, MT).T)

    in_maps = []
    for k in range(NCORES):
        sl = np.s_[:, k * PPC:(k + 1) * PPC]
        m = {}
        for name, arr in (("xs", X), ("ys", Y)):
            # pad with a large negative log so padded pixels give soft ~ 0
            s = np.full((G, PAD), -100.0, bf16)
            s[:, :PPC] = arr[sl]
            m[name] = s
        m["ohx"] = ohx
        m["ohy"] = ohy
        m["maskt"] = maskt
        m["cate"] = cate
        in_maps.append(m)
    return in_maps


def kernel(**inputs) -> np.ndarray:
    in_maps = _prep_inputs(**inputs)
    nc = _get_nc()
    import os
    trace = bool(os.environ.get("KTRACE"))
    res = run_bass_kernel_spmd(nc, in_maps, core_ids=list(range(NCORES)),
                               trace=trace)
    if trace:
        print("exec_time_ns:", res.exec_time_ns)
    return np.asarray(res.results[0]["out"], np.float32).reshape(N)


if __name__ == "__main__":
    rng = np.random.default_rng(0)
    inputs = dict(
        cate_scores=rng.random(N, np.float32),
        seg_preds_x=rng.random((G, H, W), np.float32),
        seg_preds_y=rng.random((G, H, W), np.float32),
        cate_labels=rng.integers(0, 80, N),
        x_inds=rng.integers(0, G, N),
        y_inds=rng.integers(0, G, N),
    )
    out = kernel(**inputs)
    print(out[:10])
